# revision 28
# baseline (speedup 1.0000x reference)
# Trainium2 Bass kernel for nn_ModelPositional (gnn_message_passing).
# v10: 2-way vocab x 4-way graph hybrid sharding, no collectives.
# Core c = (graph g = c//2, vocab half v = c%2). Each core runs stage 1
# for ALL 512 tokens of its graph (RWPE chain over the full 512 columns,
# masked avg), then streams its half of the projection weight from HBM
# in 512-col tiles: logits[512 tokens, 25600 cols]. Weight traffic per
# core is ~40MB (vs ~83MB for pure token sharding), which hides fully
# under the ~330us of matmul. The 17-row pe+bias chunk is resident in
# SBUF (loaded once) and its four K=17 matmuls per tile are row-packed
# into disjoint PE row-groups so they take ~one slot.

import numpy as np

B, S, KPE, V, D = 4, 512, 16, 50265, 768
NCORES = 8
P = 128
NTH = 50                # 512-wide vocab tiles per half
HW = NTH * 512          # 25600 padded cols per half
HSTART = (0, 25152)     # col offset of each half (25152 + 25600 >= 50265)
K6 = KPE + 1            # 17 rows: pe dims + bias row

_CACHE = {}


def _build_nc():
    import concourse.bacc as bacc
    import concourse.mybir as mybir
    import concourse.tile as tile
    from concourse.bass import IndirectOffsetOnAxis
    from concourse.masks import make_identity

    f32 = mybir.dt.float32
    bf16 = mybir.dt.bfloat16
    i32 = mybir.dt.int32
    Alu = mybir.AluOpType
    AX = mybir.AxisListType

    nc = bacc.Bacc(
        "TRN2",
        target_bir_lowering=False,
        debug=False,
        num_devices=NCORES,
    )

    m_in = nc.dram_tensor("m_rw", [S, S], bf16, kind="ExternalInput").ap()
    mT_in = nc.dram_tensor("m_rwT", [S, S], bf16, kind="ExternalInput").ap()
    ntmT_in = nc.dram_tensor("ntmT", [S, S], bf16, kind="ExternalInput").ap()
    codes_in = nc.dram_tensor("codes", [P, 4], i32, kind="ExternalInput").ap()
    emb_in = nc.dram_tensor("emb_table", [V, D], f32, kind="ExternalInput").ap()
    w6_in = nc.dram_tensor("w6t", [NTH * P, 6 * 512], bf16, kind="ExternalInput").ap()
    w7_in = nc.dram_tensor("w7f", [P, HW], bf16, kind="ExternalInput").ap()
    # output: per 512-col tile, contiguous [128, 4 m-chunks, 512]
    logit_out = nc.dram_tensor(
        "logit", [NTH * P, 4 * 512], bf16, kind="ExternalOutput"
    ).ap()

    with tile.TileContext(nc) as tc:
        with (
            tc.tile_pool(name="persist", bufs=1) as persist,
            tc.tile_pool(name="wp", bufs=16) as wp,
        ):
            xT_a = persist.tile([P, 6, S], bf16, name="xT_a")
            xb2 = persist.tile([P, P], bf16, name="xb2")
            ident = persist.tile([P, P], f32, name="ident")
            w7f = persist.tile([P, HW], bf16, name="w7f")

            # ---------------- Stage 1 ----------------
            with (
                tc.tile_pool(name="s1", bufs=1) as s1,
                tc.tile_pool(name="s1tmp", bufs=4) as s1t,
                tc.tile_pool(name="psA", bufs=7, space="PSUM") as psA,
            ):
                codes_sb = s1.tile([P, 4], i32, name="codes_sb")
                nc.sync.dma_start(out=codes_sb[:], in_=codes_in)
                m_sb = s1.tile([P, 4, S], bf16, name="m_sb")
                nc.sync.dma_start(
                    out=m_sb[:], in_=m_in.rearrange("(j p) s -> p j s", p=P)
                )
                # chain state Q_1 = M^T comes straight from the host — the
                # first matmul step of the power chain is skipped entirely
                qA = s1.tile([P, 4, S], bf16, name="qA")
                qB = s1.tile([P, 4, S], bf16, name="qB")
                nc.sync.dma_start(
                    out=qA[:], in_=mT_in.rearrange("(j p) s -> p j s", p=P)
                )
                ntmT_sb = s1.tile([P, 4, S], bf16, name="ntmT_sb")
                nc.sync.dma_start(
                    out=ntmT_sb[:], in_=ntmT_in.rearrange("(j p) r -> p j r", p=P)
                )
                # w7f is only needed by stage 2 — issue after stage-1 inputs
                # so it doesn't delay the chain's m_sb on the DMA queue
                nc.sync.dma_start(out=w7f[:], in_=w7_in)

                # PE warmup right before the chain: depends on m_sb so it
                # starts when the DMA lands and hands off into the chain
                # with the HAM already (or nearly) un-throttled
                for wi in range(2):
                    pw = psA.tile([P, 512], f32, tag="pq")
                    for wj in range(8):
                        nc.tensor.matmul(
                            out=pw[:, 0:P],
                            lhsT=m_sb[:, 0, 0:P],
                            rhs=m_sb[:, 0, 0:P],
                            start=(wj == 0),
                            stop=(wj == 7),
                        )

                # pe_pad cols m*32+(0..15) = diag(M^t) for token chunk m,
                # col m*32+16 = 1.0 (bias row after transpose).
                # Keep these gpsimd ops BEFORE the gathers: the chain's diag
                # writes depend on them.
                pe_pad = s1.tile([P, P], f32, name="pe_pad")
                nc.gpsimd.memset(pe_pad[:], 0.0)
                for m in range(4):
                    nc.vector.memset(pe_pad[:, m * 32 + KPE : m * 32 + KPE + 1], 1.0)
                make_identity(nc, ident[:])

                # gather embeddings for all 512 tokens of this core's graph
                emb_sb = s1.tile([P, 4, D], f32, name="emb_sb")
                for k in range(4):
                    nc.gpsimd.indirect_dma_start(
                        out=emb_sb[:, k, :],
                        out_offset=None,
                        in_=emb_in[:, :],
                        in_offset=IndirectOffsetOnAxis(
                            ap=codes_sb[:, k : k + 1], axis=0
                        ),
                    )

                # ---- RWPE chain over the full 512 columns ----
                # diag(M^1) straight from m_sb's diagonal blocks
                for i in range(4):
                    dummy = s1t.tile([P, P], f32, tag="ttr_dummy")
                    nc.vector.tensor_mul(
                        dummy[:], m_sb[:, i, i * P : (i + 1) * P], ident[:]
                    )
                    nc.vector.reduce_sum(
                        out=pe_pad[:, i * 32 : i * 32 + 1],
                        in_=dummy[:],
                        axis=AX.X,
                    )

                cur = qA
                for t in range(1, KPE):
                    nxt = qB if cur is qA else qA
                    for i in range(4):
                        pq = psA.tile([P, S], f32, tag="pq")
                        for j in range(4):
                            nc.tensor.matmul(
                                out=pq[:],
                                lhsT=m_sb[:, j, i * P : (i + 1) * P],
                                rhs=cur[:, j, :],
                                start=(j == 0),
                                stop=(j == 3),
                            )
                        # diag lives only in the [128,128] block i of pq
                        dummy = s1t.tile([P, P], f32, tag="ttr_dummy")
                        nc.vector.tensor_mul(
                            dummy[:], pq[:, i * P : (i + 1) * P], ident[:]
                        )
                        nc.vector.reduce_sum(
                            out=pe_pad[:, i * 32 + t : i * 32 + t + 1],
                            in_=dummy[:],
                            axis=AX.X,
                        )
                        if i == 1:
                            nc.vector.tensor_copy(out=nxt[:, i, :], in_=pq[:])
                        elif i == 3:
                            nc.scalar.copy(out=nxt[:, i, 0:256], in_=pq[:, 0:256])
                            nc.vector.tensor_copy(
                                out=nxt[:, i, 256:S], in_=pq[:, 256:S]
                            )
                        else:
                            nc.scalar.copy(out=nxt[:, i, :], in_=pq[:])
                    cur = nxt

                # pe chunk: one transpose [tokens, 4*32] -> [4*32, tokens];
                # rows m*32..m*32+16 are the K=17 lhsT for token chunk m
                pt = psA.tile([P, S], f32, tag="pq")
                nc.tensor.transpose(
                    out=pt[:, 0:P], in_=pe_pad[:], identity=ident[:]
                )
                nc.vector.tensor_copy(out=xb2[:], in_=pt[:, 0:P])

                # ---- emb cast + masked-average into xT_a chunks ----
                # casts go on gpsimd: scalar/vector carry the chain's critical
                # copies, and the scheduler would otherwise front-load these
                # casts (which wait on the slow gather) into their streams
                emb_bf = s1.tile([P, 4, D], bf16, name="emb_bf")
                for k in range(4):
                    nc.gpsimd.tensor_copy(out=emb_bf[:, k, :], in_=emb_sb[:, k, :])

                for w0 in (0, 3):
                    pas = {}
                    for j in range(4):
                        for c in range(w0, w0 + 3):
                            if j == 0:
                                pas[c] = psA.tile([P, S], f32, tag="pq", name=f"pa{c}")
                            nc.tensor.matmul(
                                out=pas[c][:],
                                lhsT=emb_bf[:, j, c * P : (c + 1) * P],
                                rhs=ntmT_sb[:, j, :],
                                start=(j == 0),
                                stop=(j == 3),
                            )
                    for c in range(w0, w0 + 3):
                        if c % 2 == 0:
                            nc.scalar.copy(out=xT_a[:, c, :], in_=pas[c][:])
                        else:
                            nc.vector.tensor_copy(out=xT_a[:, c, :], in_=pas[c][:])

            # ---------------- Stage 2: logits = x @ w, streamed ----------------
            with (
                tc.tile_pool(name="ob", bufs=6) as obp,
                tc.tile_pool(name="ps2", bufs=8, space="PSUM") as ps2,
            ):
                for n in range(NTH):
                    wt = wp.tile([P, 6 * 512], bf16, tag="w6")
                    nc.sync.dma_start(out=wt[:], in_=w6_in[n * P : (n + 1) * P, :])

                    po = []
                    for m in range(4):
                        pom = ps2.tile([P, 512], f32, tag="po")
                        po.append(pom)
                        for k in range(6):
                            nc.tensor.matmul(
                                out=pom[:],
                                lhsT=xT_a[:, k, m * P : (m + 1) * P],
                                rhs=wt[:, k * 512 : (k + 1) * 512],
                                start=(k == 0),
                                stop=False,
                            )
                    # K=17 pe+bias chunk, 4-way row-packed (concurrent)
                    for m in range(4):
                        nc.tensor.matmul(
                            out=po[m][:],
                            lhsT=xb2[m * 32 : m * 32 + K6, :],
                            rhs=w7f[m * 32 : m * 32 + K6, n * 512 : (n + 1) * 512],
                            start=False,
                            stop=True,
                            tile_position=(m * 32, 0),
                        )

                    ob = obp.tile([P, 4, 512], bf16, tag="ob")
                    for m in range(4):
                        if m % 2 == 0:
                            nc.scalar.copy(out=ob[:, m, :], in_=po[m][:])
                        else:
                            nc.vector.tensor_copy(out=ob[:, m, :], in_=po[m][:])
                    nc.sync.dma_start(
                        out=logit_out[n * P : (n + 1) * P, :], in_=ob[:]
                    )

    nc.compile()
    return nc


def _host_prep(code_inputs, position_idx, attn_mask, emb_table, w_lin, b_lin):
    import ml_dtypes

    bf = ml_dtypes.bfloat16
    code = np.asarray(code_inputs).astype(np.int32)
    pos = np.asarray(position_idx).astype(np.int32)
    attn = np.asarray(attn_mask).astype(np.float32)
    emb_t = np.ascontiguousarray(np.asarray(emb_table, dtype=np.float32))
    w = np.asarray(w_lin, dtype=np.float32)
    bias = np.asarray(b_lin, dtype=np.float32)

    # padded projection: rows 0..767 emb dims, 768..783 pe dims, 784 bias
    VP = HSTART[1] + HW
    wp_full = np.zeros((D + K6, VP), np.float32)
    wp_full[: D + KPE, :V] = w
    wp_full[D + KPE, :V] = bias

    w6ts, w7fs = [], []
    for v in range(2):
        wph = wp_full[:, HSTART[v] : HSTART[v] + HW].astype(bf)
        w6t = np.ascontiguousarray(
            wph[:D]
            .reshape(6, P, NTH, 512)
            .transpose(2, 1, 0, 3)
            .reshape(NTH * P, 6 * 512)
        )
        w7f = np.zeros((P, HW), bf)
        for m in range(4):
            w7f[m * 32 : m * 32 + K6] = wph[D : D + K6]
        w6ts.append(w6t)
        w7fs.append(np.ascontiguousarray(w7f))

    nodes = (pos == 0).astype(np.float32)
    token = (pos >= 2).astype(np.float32)
    eye = np.eye(S, dtype=bool)

    in_maps = []
    for c in range(NCORES):
        g, v = divmod(c, 2)
        a = attn[g]
        A = np.where(eye, 1.0, a).astype(np.float32)
        m_rw = A / A.sum(1)[:, None]

        rowsum = (a * token[g][None, :]).sum(1)
        alpha = nodes[g] / (rowsum + 1e-10)
        ntmT = a.T * token[g][:, None] * alpha[None, :]
        ntmT[eye] += 1.0 - nodes[g]

        m_bf = m_rw.astype(bf)
        in_maps.append(
            {
                "m_rw": np.ascontiguousarray(m_bf),
                "m_rwT": np.ascontiguousarray(m_bf.T),
                "ntmT": np.ascontiguousarray(ntmT.astype(bf)),
                "codes": np.ascontiguousarray(code[g].reshape(4, P).T),
                "emb_table": emb_t,
                "w6t": w6ts[v],
                "w7f": w7fs[v],
            }
        )
    return in_maps


def run(inputs, trace=False, **run_kwargs):
    from concourse.bass_utils import run_bass_kernel_spmd

    key = "nc_v10"
    nc = _CACHE.get(key)
    if nc is None:
        nc = _build_nc()
        _CACHE[key] = nc
    in_maps = _host_prep(**inputs)
    res = run_bass_kernel_spmd(
        nc, in_maps, core_ids=list(range(NCORES)), trace=trace, **run_kwargs
    )
    out = np.empty((B, S, V), np.float32)
    for c in range(NCORES):
        g, v = divmod(c, 2)
        arr = (
            res.results[c]["logit"]
            .reshape(NTH, P, 4, 512)
            .transpose(2, 1, 0, 3)
            .reshape(S, HW)
            .astype(np.float32)
        )
        lo = HSTART[v]
        hi = min(lo + HW, V)
        if v == 0:
            out[g, :, lo : HSTART[1]] = arr[:, : HSTART[1] - lo]
        else:
            out[g, :, lo:hi] = arr[:, : hi - lo]
    return out, res


def kernel(**inputs):
    logits, _ = run(inputs, trace=False)
    return logits


# revision 30
# speedup vs baseline: 1.0050x; 1.0050x over previous
# Trainium2 Bass kernel for nn_ModelPositional (gnn_message_passing).
# v10: 2-way vocab x 4-way graph hybrid sharding, no collectives.
# Core c = (graph g = c//2, vocab half v = c%2). Each core runs stage 1
# for ALL 512 tokens of its graph (RWPE chain over the full 512 columns,
# masked avg), then streams its half of the projection weight from HBM
# in 512-col tiles: logits[512 tokens, 25600 cols]. Weight traffic per
# core is ~40MB (vs ~83MB for pure token sharding), which hides fully
# under the ~330us of matmul. The 17-row pe+bias chunk is resident in
# SBUF (loaded once) and its four K=17 matmuls per tile are row-packed
# into disjoint PE row-groups so they take ~one slot.

import numpy as np

B, S, KPE, V, D = 4, 512, 16, 50265, 768
NCORES = 8
P = 128
NTH = 50                # 512-wide vocab tiles per half
HW = NTH * 512          # 25600 padded cols per half
HSTART = (0, 25152)     # col offset of each half (25152 + 25600 >= 50265)
K6 = KPE + 1            # 17 rows: pe dims + bias row

_CACHE = {}


def _build_nc():
    import concourse.bacc as bacc
    import concourse.mybir as mybir
    import concourse.tile as tile
    from concourse.bass import IndirectOffsetOnAxis
    from concourse.masks import make_identity

    f32 = mybir.dt.float32
    bf16 = mybir.dt.bfloat16
    i32 = mybir.dt.int32
    Alu = mybir.AluOpType
    AX = mybir.AxisListType

    nc = bacc.Bacc(
        "TRN2",
        target_bir_lowering=False,
        debug=False,
        num_devices=NCORES,
    )

    m_in = nc.dram_tensor("m_rw", [S, S], bf16, kind="ExternalInput").ap()
    mT_in = nc.dram_tensor("m_rwT", [S, S], bf16, kind="ExternalInput").ap()
    ntmT_in = nc.dram_tensor("ntmT", [S, S], bf16, kind="ExternalInput").ap()
    codes_in = nc.dram_tensor("codes", [P, 4], i32, kind="ExternalInput").ap()
    emb_in = nc.dram_tensor("emb_table", [V, D], f32, kind="ExternalInput").ap()
    w6_in = nc.dram_tensor("w6t", [NTH * P, 6 * 512], bf16, kind="ExternalInput").ap()
    w7_in = nc.dram_tensor("w7f", [P, HW], bf16, kind="ExternalInput").ap()
    # output: per 512-col tile, contiguous [128, 4 m-chunks, 512]
    logit_out = nc.dram_tensor(
        "logit", [NTH * P, 4 * 512], bf16, kind="ExternalOutput"
    ).ap()

    with tile.TileContext(nc) as tc:
        with (
            tc.tile_pool(name="persist", bufs=1) as persist,
            tc.tile_pool(name="wp", bufs=14) as wp,
        ):
            xT_a = persist.tile([P, 6, S], bf16, name="xT_a")
            xb2 = persist.tile([P, P], bf16, name="xb2")
            ident = persist.tile([P, P], f32, name="ident")
            w7f = persist.tile([P, HW], bf16, name="w7f")

            # ---------------- Stage 1 ----------------
            with (
                tc.tile_pool(name="s1", bufs=1) as s1,
                tc.tile_pool(name="s1tmp", bufs=3) as s1t,
                tc.tile_pool(name="psA", bufs=6, space="PSUM") as psA,
            ):
                codes_sb = s1.tile([P, 4], i32, name="codes_sb")
                nc.sync.dma_start(out=codes_sb[:], in_=codes_in)
                # chunk 0 of m_sb/qA lands first so the PE warmup and the
                # chain's first matmul group start ~2us earlier
                m_sb = s1.tile([P, 4, S], bf16, name="m_sb")
                qA = s1.tile([P, 4, S], bf16, name="qA")
                qB = s1.tile([P, 4, S], bf16, name="qB")
                m_re = m_in.rearrange("(j p) s -> p j s", p=P)
                qA_re = mT_in.rearrange("(j p) s -> p j s", p=P)
                nc.sync.dma_start(out=m_sb[:, 0, :], in_=m_re[:, 0, :])
                nc.sync.dma_start(out=qA[:, 0, :], in_=qA_re[:, 0, :])
                nc.sync.dma_start(out=m_sb[:, 1:4, :], in_=m_re[:, 1:4, :])
                nc.sync.dma_start(out=qA[:, 1:4, :], in_=qA_re[:, 1:4, :])
                ntmT_sb = s1.tile([P, 4, S], bf16, name="ntmT_sb")
                nc.sync.dma_start(
                    out=ntmT_sb[:], in_=ntmT_in.rearrange("(j p) r -> p j r", p=P)
                )
                # w7f is only needed by stage 2 — issue after stage-1 inputs
                # so it doesn't delay the chain's m_sb on the DMA queue
                nc.sync.dma_start(out=w7f[:], in_=w7_in)

                # PE warmup right before the chain: depends on m_sb so it
                # starts when the DMA lands and hands off into the chain
                # with the HAM already (or nearly) un-throttled
                for wi in range(2):
                    pw = psA.tile([P, 512], f32, tag="pq")
                    for wj in range(8):
                        nc.tensor.matmul(
                            out=pw[:, 0:P],
                            lhsT=m_sb[:, 0, 0:P],
                            rhs=m_sb[:, 0, 0:P],
                            start=(wj == 0),
                            stop=(wj == 7),
                        )

                # pe_pad cols m*32+(0..15) = diag(M^t) for token chunk m,
                # col m*32+16 = 1.0 (bias row after transpose).
                # Keep these gpsimd ops BEFORE the gathers: the chain's diag
                # writes depend on them.
                pe_pad = s1.tile([P, P], f32, name="pe_pad")
                nc.gpsimd.memset(pe_pad[:], 0.0)
                for m in range(4):
                    nc.vector.memset(pe_pad[:, m * 32 + KPE : m * 32 + KPE + 1], 1.0)
                make_identity(nc, ident[:])

                # gather embeddings for all 512 tokens of this core's graph
                emb_sb = s1.tile([P, 4, D], f32, name="emb_sb")
                for k in range(4):
                    nc.gpsimd.indirect_dma_start(
                        out=emb_sb[:, k, :],
                        out_offset=None,
                        in_=emb_in[:, :],
                        in_offset=IndirectOffsetOnAxis(
                            ap=codes_sb[:, k : k + 1], axis=0
                        ),
                    )

                # ---- RWPE chain over the full 512 columns ----
                # diag(M^1) straight from m_sb's diagonal blocks
                for i in range(4):
                    dummy = s1t.tile([P, P], f32, tag="ttr_dummy")
                    nc.vector.tensor_mul(
                        dummy[:], m_sb[:, i, i * P : (i + 1) * P], ident[:]
                    )
                    nc.vector.reduce_sum(
                        out=pe_pad[:, i * 32 : i * 32 + 1],
                        in_=dummy[:],
                        axis=AX.X,
                    )

                cur = qA
                for t in range(1, KPE):
                    nxt = qB if cur is qA else qA
                    for i in range(4):
                        pq = psA.tile([P, S], f32, tag="pq")
                        for j in range(4):
                            nc.tensor.matmul(
                                out=pq[:],
                                lhsT=m_sb[:, j, i * P : (i + 1) * P],
                                rhs=cur[:, j, :],
                                start=(j == 0),
                                stop=(j == 3),
                            )
                        # diag lives only in the [128,128] block i of pq
                        dummy = s1t.tile([P, P], f32, tag="ttr_dummy")
                        nc.vector.tensor_mul(
                            dummy[:], pq[:, i * P : (i + 1) * P], ident[:]
                        )
                        nc.vector.reduce_sum(
                            out=pe_pad[:, i * 32 + t : i * 32 + t + 1],
                            in_=dummy[:],
                            axis=AX.X,
                        )
                        if i == 1:
                            nc.vector.tensor_copy(out=nxt[:, i, :], in_=pq[:])
                        else:
                            nc.scalar.copy(out=nxt[:, i, :], in_=pq[:])
                    cur = nxt

                # pe chunk: one transpose [tokens, 4*32] -> [4*32, tokens];
                # rows m*32..m*32+16 are the K=17 lhsT for token chunk m
                pt = psA.tile([P, S], f32, tag="pq")
                nc.tensor.transpose(
                    out=pt[:, 0:P], in_=pe_pad[:], identity=ident[:]
                )
                nc.vector.tensor_copy(out=xb2[:], in_=pt[:, 0:P])

                # ---- emb cast + masked-average into xT_a chunks ----
                # casts go on gpsimd: scalar/vector carry the chain's critical
                # copies, and the scheduler would otherwise front-load these
                # casts (which wait on the slow gather) into their streams
                emb_bf = s1.tile([P, 4, D], bf16, name="emb_bf")
                for k in range(4):
                    nc.gpsimd.tensor_copy(out=emb_bf[:, k, :], in_=emb_sb[:, k, :])

                for w0 in (0, 3):
                    pas = {}
                    for j in range(4):
                        for c in range(w0, w0 + 3):
                            if j == 0:
                                pas[c] = psA.tile([P, S], f32, tag="pq", name=f"pa{c}")
                            nc.tensor.matmul(
                                out=pas[c][:],
                                lhsT=emb_bf[:, j, c * P : (c + 1) * P],
                                rhs=ntmT_sb[:, j, :],
                                start=(j == 0),
                                stop=(j == 3),
                            )
                    for c in range(w0, w0 + 3):
                        if c % 2 == 0:
                            nc.scalar.copy(out=xT_a[:, c, :], in_=pas[c][:])
                        else:
                            nc.vector.tensor_copy(out=xT_a[:, c, :], in_=pas[c][:])

            # ---------------- Stage 2: logits = x @ w, streamed ----------------
            with (
                tc.tile_pool(name="ob", bufs=4) as obp,
                tc.tile_pool(name="ps2", bufs=8, space="PSUM") as ps2,
            ):
                for n in range(NTH):
                    wt = wp.tile([P, 6 * 512], bf16, tag="w6")
                    nc.sync.dma_start(out=wt[:], in_=w6_in[n * P : (n + 1) * P, :])

                    po = []
                    for m in range(4):
                        pom = ps2.tile([P, 512], f32, tag="po")
                        po.append(pom)
                        for k in range(6):
                            nc.tensor.matmul(
                                out=pom[:],
                                lhsT=xT_a[:, k, m * P : (m + 1) * P],
                                rhs=wt[:, k * 512 : (k + 1) * 512],
                                start=(k == 0),
                                stop=False,
                            )
                    # K=17 pe+bias chunk, 4-way row-packed (concurrent)
                    for m in range(4):
                        nc.tensor.matmul(
                            out=po[m][:],
                            lhsT=xb2[m * 32 : m * 32 + K6, :],
                            rhs=w7f[m * 32 : m * 32 + K6, n * 512 : (n + 1) * 512],
                            start=False,
                            stop=True,
                            tile_position=(m * 32, 0),
                        )

                    ob = obp.tile([P, 4, 512], bf16, tag="ob")
                    for m in range(4):
                        if m % 2 == 0:
                            nc.scalar.copy(out=ob[:, m, :], in_=po[m][:])
                        else:
                            nc.vector.tensor_copy(out=ob[:, m, :], in_=po[m][:])
                    nc.sync.dma_start(
                        out=logit_out[n * P : (n + 1) * P, :], in_=ob[:]
                    )

    nc.compile()
    return nc


def _host_prep(code_inputs, position_idx, attn_mask, emb_table, w_lin, b_lin):
    import ml_dtypes

    bf = ml_dtypes.bfloat16
    code = np.asarray(code_inputs).astype(np.int32)
    pos = np.asarray(position_idx).astype(np.int32)
    attn = np.asarray(attn_mask).astype(np.float32)
    emb_t = np.ascontiguousarray(np.asarray(emb_table, dtype=np.float32))
    w = np.asarray(w_lin, dtype=np.float32)
    bias = np.asarray(b_lin, dtype=np.float32)

    # padded projection: rows 0..767 emb dims, 768..783 pe dims, 784 bias
    VP = HSTART[1] + HW
    wp_full = np.zeros((D + K6, VP), np.float32)
    wp_full[: D + KPE, :V] = w
    wp_full[D + KPE, :V] = bias

    w6ts, w7fs = [], []
    for v in range(2):
        wph = wp_full[:, HSTART[v] : HSTART[v] + HW].astype(bf)
        w6t = np.ascontiguousarray(
            wph[:D]
            .reshape(6, P, NTH, 512)
            .transpose(2, 1, 0, 3)
            .reshape(NTH * P, 6 * 512)
        )
        w7f = np.zeros((P, HW), bf)
        for m in range(4):
            w7f[m * 32 : m * 32 + K6] = wph[D : D + K6]
        w6ts.append(w6t)
        w7fs.append(np.ascontiguousarray(w7f))

    nodes = (pos == 0).astype(np.float32)
    token = (pos >= 2).astype(np.float32)
    eye = np.eye(S, dtype=bool)

    in_maps = []
    for c in range(NCORES):
        g, v = divmod(c, 2)
        a = attn[g]
        A = np.where(eye, 1.0, a).astype(np.float32)
        m_rw = A / A.sum(1)[:, None]

        rowsum = (a * token[g][None, :]).sum(1)
        alpha = nodes[g] / (rowsum + 1e-10)
        ntmT = a.T * token[g][:, None] * alpha[None, :]
        ntmT[eye] += 1.0 - nodes[g]

        m_bf = m_rw.astype(bf)
        in_maps.append(
            {
                "m_rw": np.ascontiguousarray(m_bf),
                "m_rwT": np.ascontiguousarray(m_bf.T),
                "ntmT": np.ascontiguousarray(ntmT.astype(bf)),
                "codes": np.ascontiguousarray(code[g].reshape(4, P).T),
                "emb_table": emb_t,
                "w6t": w6ts[v],
                "w7f": w7fs[v],
            }
        )
    return in_maps


def run(inputs, trace=False, **run_kwargs):
    from concourse.bass_utils import run_bass_kernel_spmd

    key = "nc_v10"
    nc = _CACHE.get(key)
    if nc is None:
        nc = _build_nc()
        _CACHE[key] = nc
    in_maps = _host_prep(**inputs)
    res = run_bass_kernel_spmd(
        nc, in_maps, core_ids=list(range(NCORES)), trace=trace, **run_kwargs
    )
    out = np.empty((B, S, V), np.float32)
    for c in range(NCORES):
        g, v = divmod(c, 2)
        arr = (
            res.results[c]["logit"]
            .reshape(NTH, P, 4, 512)
            .transpose(2, 1, 0, 3)
            .reshape(S, HW)
            .astype(np.float32)
        )
        lo = HSTART[v]
        hi = min(lo + HW, V)
        if v == 0:
            out[g, :, lo : HSTART[1]] = arr[:, : HSTART[1] - lo]
        else:
            out[g, :, lo:hi] = arr[:, : hi - lo]
    return out, res


def kernel(**inputs):
    logits, _ = run(inputs, trace=False)
    return logits
